# revision 4
# baseline (speedup 1.0000x reference)
"""Differentiable persistence landscape kernel for Trainium2 (Bass/Tile).

Computes, for each (batch, homology-dim) persistence diagram and each t on a
256-point grid, the softmax-weighted sum of the 5 largest tent-function
heights min(t - birth, death - t) clamped at 0 over 2048 diagram points.

Strategy (8 NeuronCores, pure data parallelism over the batch axis):
  - host: m = (b+d)/2, h = (d-b)/2 in f64, each split into 3 bf16 terms
    (hi/mid/lo) whose f32 sum reconstructs the f32 value exactly.
  - PE: broadcast m,h across 128 partitions (t values) with K=3 bf16
    matmuls against an all-ones [3,128] lhsT -> PSUM (exact, 1 cycle/row).
  - ACT: A = Abs(t - m) using per-partition bias t, reading PSUM.
  - DVE: v = h - A, then InstMax -> top-8 per t-row (sorted desc, keeps
    duplicates == top_k semantics).
  - clamp-at-0 is applied to the top-5 values after selection (monotone
    transforms commute with order statistics), then a weighted sum with
    softmax(landscape_weights) * persistence_scale.
"""

import sys

for _p in ("/opt/trn_rl_repo", "/root/.axon_site/_ro/trn_rl_repo"):
    if _p not in sys.path:
        sys.path.insert(0, _p)

from contextlib import ExitStack

import ml_dtypes
import numpy as np

import concourse.bass as bass
import concourse.tile as tile
from concourse import bacc
from concourse import mybir
from concourse.bass_utils import run_bass_kernel_spmd

# Problem constants (hardcoded per contract)
B, D, P = 64, 3, 2048
RES = 256
MAX_PERS = 2.0
K = 5
N_CORES = 8
BS = B // N_CORES          # batches per core
NS = BS * D                # diagram slices per core (24)
CHUNK = 1024               # point-chunk so PSUM double-buffers (2 banks/tile)

f32 = mybir.dt.float32
bf16 = mybir.dt.bfloat16


def _build_kernel_body(ctx: ExitStack, tc: tile.TileContext,
                       out_ap: bass.AP, mh_ap: bass.AP,
                       tcols_ap: bass.AP, w120_ap: bass.AP):
    """Emit the per-core program.

    out_ap:   [2, 128, NS] f32   (j, r, slice) -> landscape value at t=128j+r
    mh_ap:    [NS, 2, 3, P] bf16 (slice, m/h, hi/mid/lo term, point)
    tcols_ap: [128, 2] f32       column j holds t[128j : 128j+128]
    w120_ap:  [3, 120] bf16      softmax(w)*scale terms, tiled NS.. wait 24x5
    """
    nc = tc.nc

    const_pool = ctx.enter_context(tc.tile_pool(name="const", bufs=1))
    in_pool = ctx.enter_context(tc.tile_pool(name="inp", bufs=4))
    psum_pool = ctx.enter_context(tc.tile_pool(name="ps", bufs=2, space="PSUM"))
    a_pool = ctx.enter_context(tc.tile_pool(name="abs", bufs=4))
    v_pool = ctx.enter_context(tc.tile_pool(name="v", bufs=2))
    col_pool = ctx.enter_context(tc.tile_pool(name="col", bufs=1))
    tail_pool = ctx.enter_context(tc.tile_pool(name="tail", bufs=1))

    ones3 = const_pool.tile([3, 128], bf16, tag="ones3")
    nc.vector.memset(ones3[:], 1.0)

    t_sb = const_pool.tile([128, 2], f32, tag="tsb")
    nc.sync.dma_start(t_sb[:], tcols_ap)

    w3_sb = const_pool.tile([3, 120], bf16, tag="w3")
    nc.sync.dma_start(w3_sb[:], w120_ap)

    # broadcast the 24x5 weight pattern across all 128 partitions via PE
    w_psum = psum_pool.tile([128, CHUNK], f32, tag="psm")
    nc.tensor.matmul(w_psum[:, :120], lhsT=ones3[:], rhs=w3_sb[:],
                     start=True, stop=True)
    w_sb = const_pool.tile([128, 120], f32, tag="wsb")
    nc.scalar.activation(w_sb[:], w_psum[:, :120],
                         mybir.ActivationFunctionType.Copy)

    cols = [col_pool.tile([128, NS * 8], f32, tag=f"col{j}", name=f"col{j}")
            for j in range(2)]

    for i in range(NS):
        m3 = in_pool.tile([3, P], bf16, tag="m3")
        nc.sync.dma_start(m3[:], mh_ap[i, 0])
        h3 = in_pool.tile([3, P], bf16, tag="h3")
        nc.sync.dma_start(h3[:], mh_ap[i, 1])

        vts = [v_pool.tile([128, P], f32, tag=f"v{j}", name=f"v{j}")
               for j in range(2)]

        for c in range(P // CHUNK):
            pm = psum_pool.tile([128, CHUNK], f32, tag="psm")
            ph = psum_pool.tile([128, CHUNK], f32, tag="psh")
            for s in range(CHUNK // 512):
                lo = c * CHUNK + s * 512
                nc.tensor.matmul(pm[:, s * 512:(s + 1) * 512], lhsT=ones3[:],
                                 rhs=m3[:, lo:lo + 512], start=True, stop=True)
                nc.tensor.matmul(ph[:, s * 512:(s + 1) * 512], lhsT=ones3[:],
                                 rhs=h3[:, lo:lo + 512], start=True, stop=True)
            for j in range(2):
                at = a_pool.tile([128, CHUNK], f32, tag="A")
                nc.scalar.activation(at[:], pm[:],
                                     mybir.ActivationFunctionType.Abs,
                                     bias=t_sb[:, j:j + 1], scale=-1.0)
                nc.vector.tensor_tensor(vts[j][:, c * CHUNK:(c + 1) * CHUNK],
                                        ph[:], at[:],
                                        mybir.AluOpType.subtract)

        for j in range(2):
            nc.vector.max(out=cols[j][:, i * 8:(i + 1) * 8], in_=vts[j][:])

    # tail: relu + weighted sum over the 5 largest, batched over all slices
    for j in range(2):
        rl = tail_pool.tile([128, NS * 8], f32, tag="rl")
        nc.vector.tensor_scalar_max(rl[:], cols[j][:], 0.0)
        prod = tail_pool.tile([128, NS * K], f32, tag="prod")
        rl3 = rl[:].rearrange("p (i e) -> p i e", e=8)[:, :, 0:K]
        w3v = w_sb[:].rearrange("p (i e) -> p i e", e=K)
        prod3 = prod[:].rearrange("p (i e) -> p i e", e=K)
        nc.vector.tensor_tensor(prod3, rl3, w3v, mybir.AluOpType.mult)
        osb = tail_pool.tile([128, NS], f32, tag="osb")
        nc.vector.reduce_sum(osb[:], prod3, axis=mybir.AxisListType.X)
        nc.sync.dma_start(out_ap[j], osb[:])


def build_nc():
    nc = bacc.Bacc("TRN2", target_bir_lowering=False, debug=False,
                   enable_asserts=False, num_devices=N_CORES)
    mh_t = nc.dram_tensor("mh", [NS, 2, 3, P], bf16, kind="ExternalInput")
    tcols_t = nc.dram_tensor("tcols", [128, 2], f32, kind="ExternalInput")
    w120_t = nc.dram_tensor("w120", [3, 120], bf16, kind="ExternalInput")
    out_t = nc.dram_tensor("out", [2, 128, NS], f32, kind="ExternalOutput")
    with tile.TileContext(nc) as tc:
        with ExitStack() as ctx:
            _build_kernel_body(ctx, tc, out_t.ap(), mh_t.ap(),
                               tcols_t.ap(), w120_t.ap())
    nc.compile()
    return nc


def _split3_bf16(x64: np.ndarray) -> np.ndarray:
    """Split f32(x64) into 3 bf16 terms whose f32 sum reconstructs it
    exactly. Returns [..., 3] stacked on a new last axis."""
    x = x64.astype(np.float32)
    hi = x.astype(ml_dtypes.bfloat16)
    r1 = x - hi.astype(np.float32)
    mid = r1.astype(ml_dtypes.bfloat16)
    r2 = r1 - mid.astype(np.float32)
    lo = r2.astype(ml_dtypes.bfloat16)
    return np.stack([hi, mid, lo], axis=-1)


def make_inputs(births: np.ndarray, deaths: np.ndarray,
                landscape_weights: np.ndarray, persistence_scale: np.ndarray):
    """Host-side marshalling: per-core input maps."""
    births = np.asarray(births, np.float32)
    deaths = np.asarray(deaths, np.float32)
    lw = np.asarray(landscape_weights, np.float32)
    scale = float(np.asarray(persistence_scale, np.float32))

    m64 = (births.astype(np.float64) + deaths.astype(np.float64)) * 0.5
    h64 = (deaths.astype(np.float64) - births.astype(np.float64)) * 0.5
    msp = _split3_bf16(m64)                      # [B, D, P, 3]
    hsp = _split3_bf16(h64)
    # -> [B*D, 2, 3, P]
    mh = np.stack([msp, hsp], axis=-2)           # [B, D, P, 2, 3]
    mh = np.ascontiguousarray(
        mh.reshape(B * D, P, 2, 3).transpose(0, 2, 3, 1))

    t = np.linspace(0.0, MAX_PERS, RES).astype(np.float32)
    tcols = np.ascontiguousarray(t.reshape(2, 128).T)

    e = np.exp(lw - lw.max())
    w = (e / e.sum()).astype(np.float32) * scale
    w3 = _split3_bf16(w.astype(np.float64)).T    # [3, K]
    w120 = np.ascontiguousarray(np.tile(w3, (1, NS)))

    shards = mh.reshape(N_CORES, NS, 2, 3, P)
    return [{"mh": np.ascontiguousarray(shards[c]),
             "tcols": tcols, "w120": w120} for c in range(N_CORES)]


def gather_output(results) -> np.ndarray:
    outs = []
    for c in range(N_CORES):
        arr = results[c]["out"]                  # [2, 128, NS]
        outs.append(np.transpose(arr, (2, 0, 1)).reshape(NS, RES))
    return np.concatenate(outs, axis=0).reshape(B, D, RES).astype(np.float32)


_NC_CACHE = {}


def kernel(births, deaths, landscape_weights, persistence_scale,
           **run_kwargs) -> np.ndarray:
    in_maps = make_inputs(births, deaths, landscape_weights,
                          persistence_scale)
    if "nc" not in _NC_CACHE:
        _NC_CACHE["nc"] = build_nc()
    res = run_bass_kernel_spmd(_NC_CACHE["nc"], in_maps,
                               core_ids=list(range(N_CORES)), **run_kwargs)
    out = gather_output(res.results)
    if run_kwargs:
        kernel.last_results = res
    return out


if __name__ == "__main__":
    rng = np.random.default_rng(0)
    b = rng.random((B, D, P), dtype=np.float32)
    d = b + 0.02 + rng.random((B, D, P), dtype=np.float32)
    out = kernel(b, d, np.ones(K, np.float32), np.float32(1.0))
    print("kernel ran, out shape:", out.shape, out.dtype)
